# revision 4
# baseline (speedup 1.0000x reference)
"""GCN layer (gather + segment_sum + linear + relu) as a Trainium2 Bass kernel.

Math: out = relu(segment_sum(x[src], dst) @ W + b)
    = relu((A^T x) @ W + b)   where A[s, d] = #edges s -> d  (dense count matrix)

Strategy (8 cores, no collectives):
  - Shard destination nodes across cores (1250 nodes/core -> 1280 padded cols).
  - Host builds the per-core dense count matrix A_c [10112, 1280] (bf16; counts
    are small ints, exact) plus an error-compensated bf16 split of x
    (x = x_hi + x_lo), so the segment-sum runs on the PE array at bf16 speed
    with ~fp32 accuracy (fp32 PSUM accumulation).
  - Device per core: H^T[k, d] = sum_s x[s, k] * A[s, d] over 79 src tiles
    (x tiles stationary, A streamed), then out^T = relu(W^T @ H^T + b) with
    fp32 matmul; bias+relu fused in one ScalarE activation op.
  - Host transposes/concats the 8 [128, 1280] outputs.
"""

import numpy as np
import ml_dtypes

N_NODES = 10000
N_EDGES = 640000
D = 128
NCORES = 8
NPC = N_NODES // NCORES            # 1250 dst nodes per core
DCOLS = 1280                       # padded dst cols per core (10 tiles of 128)
STILES = 79                        # ceil(10000 / 128) src tiles
SPAD = STILES * 128                # 10112 padded src rows
GROUPS = [(0, 512), (512, 512), (1024, 256)]   # dst col groups (PSUM banks)
ACH = 4                            # src tiles per A DMA chunk
XCH = 16                           # src tiles per x DMA chunk

BF16 = ml_dtypes.bfloat16

_prog_cache = {}


def _build_program():
    from concourse import mybir
    import concourse.bacc as bacc
    import concourse.tile as tile

    # Bacc (not raw Bass): its compile pipeline legalizes multi-wait
    # instructions via event semaphores; raw Bass programs fail walrus
    # codegen with "Too many sync wait commands".
    nc = bacc.Bacc("TRN2", target_bir_lowering=False)

    xhl = nc.dram_tensor("xhl", [SPAD, 2 * D], mybir.dt.bfloat16, kind="ExternalInput")
    A = nc.dram_tensor("A", [SPAD, DCOLS], mybir.dt.bfloat16, kind="ExternalInput")
    W = nc.dram_tensor("W", [D, D], mybir.dt.float32, kind="ExternalInput")
    bcol = nc.dram_tensor("bcol", [D, 1], mybir.dt.float32, kind="ExternalInput")
    outT = nc.dram_tensor("outT", [D, DCOLS], mybir.dt.float32, kind="ExternalOutput")

    # HBM views with the src-tile index split out: row (s p) -> [p, s, cols]
    xhl_r = xhl.rearrange("(s p) d -> p s d", p=128)
    A_r = A.rearrange("(s p) d -> p s d", p=128)

    f32 = mybir.dt.float32
    Relu = mybir.ActivationFunctionType.Relu

    with tile.TileContext(nc) as tc:
        with (
            tc.tile_pool(name="xpool", bufs=1) as xpool,
            tc.tile_pool(name="apool", bufs=6) as apool,
            tc.tile_pool(name="cpool", bufs=1) as cpool,
            tc.tile_pool(name="hpool", bufs=2) as hpool,
            tc.tile_pool(name="opool", bufs=2) as opool,
            tc.tile_pool(name="pspool", bufs=1, space="PSUM") as pspool,
            tc.tile_pool(name="ps2pool", bufs=2, space="PSUM") as ps2pool,
        ):
            # ---- constants ----
            w_sb = cpool.tile([D, D], f32, tag="w")
            nc.sync.dma_start(out=w_sb[:], in_=W[:, :])
            b_sb = cpool.tile([D, 1], f32, tag="b")
            nc.sync.dma_start(out=b_sb[:], in_=bcol[:, :])

            # ---- resident x_hi/x_lo tiles (interleaved rows: [hi | lo]) ----
            x_tiles = []  # per src tile: AP [128, 256] (cols 0:128 hi, 128:256 lo)
            for c0 in range(0, STILES, XCH):
                n = min(XCH, STILES - c0)
                t = xpool.tile([128, n, 2 * D], mybir.dt.bfloat16, tag=f"xhl{c0}")
                nc.sync.dma_start(out=t[:], in_=xhl_r[:, c0 : c0 + n, :])
                for i in range(n):
                    x_tiles.append(t[:, i, :])

            # ---- phase 1: H^T[k, d] accumulation per col group ----
            ps = []
            for g, (off, wdt) in enumerate(GROUPS):
                pt = pspool.tile([128, wdt], f32, tag=f"ps{g}")
                ps.append(pt)

            def phase1(g):
                off, wdt = GROUPS[g]
                nmm = 0
                total_mm = 2 * STILES
                for s0 in range(0, STILES, ACH):
                    n = min(ACH, STILES - s0)
                    at = apool.tile([128, n, wdt], mybir.dt.bfloat16, tag="a")
                    nc.sync.dma_start(
                        out=at[:], in_=A_r[:, s0 : s0 + n, off : off + wdt]
                    )
                    for i in range(n):
                        s = s0 + i
                        for half in (0, 1):  # hi, lo
                            nc.tensor.matmul(
                                out=ps[g][:],
                                lhsT=x_tiles[s][:, half * D : (half + 1) * D],
                                rhs=at[:, i, :],
                                start=(nmm == 0),
                                stop=(nmm == total_mm - 1),
                            )
                            nmm += 1

            def phase2(g):
                off, wdt = GROUPS[g]
                hT = hpool.tile([128, 512], f32, tag="hT")
                nc.vector.tensor_copy(out=hT[:, :wdt], in_=ps[g][:])
                po = ps2pool.tile([128, 512], f32, tag="po")
                nc.tensor.matmul(
                    out=po[:, :wdt], lhsT=w_sb[:], rhs=hT[:, :wdt],
                    start=True, stop=True,
                )
                ot = opool.tile([128, 512], f32, tag="ot")
                nc.scalar.activation(
                    out=ot[:, :wdt], in_=po[:, :wdt], func=Relu, bias=b_sb[:], scale=1.0
                )
                nc.sync.dma_start(out=outT[:, off : off + wdt], in_=ot[:, :wdt])

            # order: ph1(0), ph1(1), ph2(0), ph1(2), ph2(1), ph2(2)
            phase1(0)
            phase1(1)
            phase2(0)
            phase1(2)
            phase2(1)
            phase2(2)

    nc.finalize()
    return nc


def _host_preprocess(x, src, dst, W, b):
    x = np.asarray(x, dtype=np.float32)
    xh = x.astype(BF16)
    xl = (x - xh.astype(np.float32)).astype(BF16)
    xhl = np.zeros((SPAD, 2 * D), dtype=BF16)
    xhl[:N_NODES, :D] = xh
    xhl[:N_NODES, D:] = xl

    src = np.asarray(src).astype(np.int64)
    dst = np.asarray(dst).astype(np.int64)

    A_mats = []
    for c in range(NCORES):
        lo, hi = c * NPC, (c + 1) * NPC
        m = (dst >= lo) & (dst < hi)
        idx = src[m] * DCOLS + (dst[m] - lo)
        cnt = np.bincount(idx, minlength=SPAD * DCOLS)
        A_mats.append(cnt.reshape(SPAD, DCOLS).astype(BF16))

    Wf = np.asarray(W, dtype=np.float32)
    bc = np.asarray(b, dtype=np.float32).reshape(D, 1)
    return xhl, A_mats, Wf, bc


def kernel(x, src, dst, W, b):
    from concourse.bass_utils import run_bass_kernel_spmd

    xhl, A_mats, Wf, bc = _host_preprocess(x, src, dst, W, b)

    if "nc" not in _prog_cache:
        _prog_cache["nc"] = _build_program()
    nc = _prog_cache["nc"]

    in_maps = [
        {"xhl": xhl, "A": A_mats[c], "W": Wf, "bcol": bc} for c in range(NCORES)
    ]
    res = run_bass_kernel_spmd(nc, in_maps, core_ids=list(range(NCORES)))

    out = np.empty((N_NODES, D), dtype=np.float32)
    for c in range(NCORES):
        outT = res.results[c]["outT"]  # [128, 1280]
        out[c * NPC : (c + 1) * NPC] = outT[:, :NPC].T
    return out


# revision 5
# speedup vs baseline: 1.1419x; 1.1419x over previous
"""GCN layer (gather + segment_sum + linear + relu) as a Trainium2 Bass kernel.

Math: out = relu(segment_sum(x[src], dst) @ W + b)
    = relu((A^T x) @ W + b)   where A[s, d] = #edges s -> d  (dense count matrix)

Strategy (8 cores, no collectives):
  - Shard destination nodes across cores (1250 dst nodes per core).
  - Host builds the per-core dense count matrix A_c [10112, 1250] in fp8e4
    (counts are small ints <= 16, exact in e4m3) plus an error-compensated
    bf16 split of x (x = x_hi + x_lo), so the segment-sum runs on the PE
    array with ~fp32 accuracy (fp32 PSUM accumulation, two bf16 passes).
  - Device per core: H^T[k, d] = sum_s x[s, k] * A[s, d] over 79 src tiles
    (x tiles stationary bf16, A streamed fp8), then out^T = relu(W^T H^T + b)
    with an fp32 matmul; bias+relu fused in one ScalarE activation op.
  - DMA is spread over both HWDGE queues (sync: A stream, scalar: x/out)
    because DMA_DIRECT2D occupies the issuing engine for the whole transfer.
  - Host transposes/concats the 8 [128, 1250] outputs.
"""

import numpy as np
import ml_dtypes

N_NODES = 10000
N_EDGES = 640000
D = 128
NCORES = 8
NPC = N_NODES // NCORES            # 1250 dst nodes per core
DCOLS = NPC                        # dst cols per core (exact)
STILES = 79                        # ceil(10000 / 128) src tiles
SPAD = STILES * 128                # 10112 padded src rows
GROUPS = [(0, 512), (512, 512), (1024, 226)]   # dst col groups (PSUM banks)
ACH = 4                            # src tiles per A DMA chunk
XCHUNKS = [4, 15, 20, 20, 20]      # x DMA chunk sizes (small first: PE ramps fast)

BF16 = ml_dtypes.bfloat16
FP8 = ml_dtypes.float8_e4m3

_prog_cache = {}


def _build_program():
    from concourse import mybir
    import concourse.bacc as bacc
    import concourse.tile as tile

    # Bacc (not raw Bass): its compile pipeline legalizes multi-wait
    # instructions via event semaphores; raw Bass programs fail walrus
    # codegen with "Too many sync wait commands".
    nc = bacc.Bacc("TRN2", target_bir_lowering=False)

    xhl = nc.dram_tensor("xhl", [SPAD, 2 * D], mybir.dt.bfloat16, kind="ExternalInput")
    A = nc.dram_tensor("A", [SPAD, DCOLS], mybir.dt.float8e4, kind="ExternalInput")
    W = nc.dram_tensor("W", [D, D], mybir.dt.float32, kind="ExternalInput")
    bcol = nc.dram_tensor("bcol", [D, 1], mybir.dt.float32, kind="ExternalInput")
    outT = nc.dram_tensor("outT", [D, DCOLS], mybir.dt.float32, kind="ExternalOutput")

    # HBM views with the src-tile index split out: row (s p) -> [p, s, cols]
    xhl_r = xhl.rearrange("(s p) d -> p s d", p=128)
    A_r = A.rearrange("(s p) d -> p s d", p=128)

    f32 = mybir.dt.float32
    Relu = mybir.ActivationFunctionType.Relu

    with tile.TileContext(nc) as tc:
        with (
            tc.tile_pool(name="xpool", bufs=1) as xpool,
            tc.tile_pool(name="apool", bufs=8) as apool,
            tc.tile_pool(name="cpool", bufs=1) as cpool,
            tc.tile_pool(name="hpool", bufs=2) as hpool,
            tc.tile_pool(name="opool", bufs=2) as opool,
            tc.tile_pool(name="pspool", bufs=1, space="PSUM") as pspool,
            tc.tile_pool(name="ps2pool", bufs=2, space="PSUM") as ps2pool,
        ):
            # ---- resident x_hi/x_lo tiles (interleaved rows: [hi | lo]) ----
            # loaded on the scalar HWDGE queue; first chunk small so the PE
            # can start after ~300KB instead of ~5MB
            x_tiles = []  # per src tile: AP [128, 256] (cols 0:128 hi, 128:256 lo)
            c0 = 0
            for ci, n in enumerate(XCHUNKS):
                n = min(n, STILES - c0)
                if n <= 0:
                    break
                t = xpool.tile([128, n, 2 * D], mybir.dt.bfloat16, tag=f"xhl{ci}")
                nc.scalar.dma_start(out=t[:], in_=xhl_r[:, c0 : c0 + n, :])
                for i in range(n):
                    x_tiles.append(t[:, i, :])
                c0 += n

            # ---- constants (scalar queue, after the first x chunk) ----
            w_sb = cpool.tile([D, D], f32, tag="w")
            nc.scalar.dma_start(out=w_sb[:], in_=W[:, :])
            b_sb = cpool.tile([D, 1], f32, tag="b")
            nc.scalar.dma_start(out=b_sb[:], in_=bcol[:, :])

            # ---- phase 1: H^T[k, d] accumulation per col group ----
            ps = []
            for g, (off, wdt) in enumerate(GROUPS):
                pt = pspool.tile([128, wdt], f32, tag=f"ps{g}")
                ps.append(pt)

            def phase1(g):
                off, wdt = GROUPS[g]
                nmm = 0
                total_mm = 2 * STILES
                for s0 in range(0, STILES, ACH):
                    n = min(ACH, STILES - s0)
                    at = apool.tile([128, n, wdt], mybir.dt.float8e4, tag="a")
                    nc.sync.dma_start(
                        out=at[:], in_=A_r[:, s0 : s0 + n, off : off + wdt]
                    )
                    for i in range(n):
                        s = s0 + i
                        for half in (0, 1):  # hi, lo
                            nc.tensor.matmul(
                                out=ps[g][:],
                                lhsT=x_tiles[s][:, half * D : (half + 1) * D],
                                rhs=at[:, i, :],
                                start=(nmm == 0),
                                stop=(nmm == total_mm - 1),
                            )
                            nmm += 1

            def phase2(g):
                off, wdt = GROUPS[g]
                hT = hpool.tile([128, wdt], f32, tag="hT")
                nc.vector.tensor_copy(out=hT[:], in_=ps[g][:])
                po = ps2pool.tile([128, wdt], f32, tag="po")
                nc.tensor.matmul(out=po[:], lhsT=w_sb[:], rhs=hT[:], start=True, stop=True)
                ot = opool.tile([128, wdt], f32, tag="ot")
                nc.scalar.activation(out=ot[:], in_=po[:], func=Relu, bias=b_sb[:], scale=1.0)
                nc.scalar.dma_start(out=outT[:, off : off + wdt], in_=ot[:])

            # order: ph1(0), ph1(1), ph2(0), ph1(2), ph2(1), ph2(2)
            phase1(0)
            phase1(1)
            phase2(0)
            phase1(2)
            phase2(1)
            phase2(2)

    nc.finalize()
    return nc


def _host_preprocess(x, src, dst, W, b):
    x = np.asarray(x, dtype=np.float32)
    xh = x.astype(BF16)
    xl = (x - xh.astype(np.float32)).astype(BF16)
    xhl = np.zeros((SPAD, 2 * D), dtype=BF16)
    xhl[:N_NODES, :D] = xh
    xhl[:N_NODES, D:] = xl

    src = np.asarray(src).astype(np.int64)
    dst = np.asarray(dst).astype(np.int64)

    A_mats = []
    for c in range(NCORES):
        lo, hi = c * NPC, (c + 1) * NPC
        m = (dst >= lo) & (dst < hi)
        idx = src[m] * DCOLS + (dst[m] - lo)
        cnt = np.bincount(idx, minlength=SPAD * DCOLS)
        assert cnt.max() <= 16, "count too large for exact fp8e4"
        A_mats.append(cnt.reshape(SPAD, DCOLS).astype(FP8))

    Wf = np.asarray(W, dtype=np.float32)
    bc = np.asarray(b, dtype=np.float32).reshape(D, 1)
    return xhl, A_mats, Wf, bc


def kernel(x, src, dst, W, b):
    from concourse.bass_utils import run_bass_kernel_spmd

    xhl, A_mats, Wf, bc = _host_preprocess(x, src, dst, W, b)

    if "nc" not in _prog_cache:
        _prog_cache["nc"] = _build_program()
    nc = _prog_cache["nc"]

    in_maps = [
        {"xhl": xhl, "A": A_mats[c], "W": Wf, "bcol": bc} for c in range(NCORES)
    ]
    res = run_bass_kernel_spmd(nc, in_maps, core_ids=list(range(NCORES)))

    out = np.empty((N_NODES, D), dtype=np.float32)
    for c in range(NCORES):
        outT = res.results[c]["outT"]  # [128, 1250]
        out[c * NPC : (c + 1) * NPC] = outT.T
    return out


# revision 9
# speedup vs baseline: 1.3890x; 1.2164x over previous
"""GCN layer (gather + segment_sum + linear + relu) as a Trainium2 Bass kernel.

Math: out = relu(segment_sum(x[src], dst) @ W + b)
    = relu((A^T x) @ W + b)   where A[s, d] = #edges s -> d  (dense count matrix)

Strategy (8 cores, no collectives):
  - Shard destination nodes across cores (1250 dst nodes per core).
  - Host builds the per-core dense count matrix A_c [10112, 1250] in fp8e4
    (counts are small ints <= 16, exact in e4m3) and an error-compensated
    split of x:  x ~= x_hi (bf16) + x_lo8/512 (fp8e4, the bf16 residual
    scaled by 512). Segment-sum runs on the PE array in two passes:
      hi: bf16 stationary x_hi  X  fp8 A (1 col/cycle)
      lo: fp8 DoubleRow - x_lo8 pairs X A pairs (2 contraction rows/cycle)
    accumulating in separate fp32 PSUM banks; combined as
    H = ps_hi + ps_lo/512 (ScalarE scale-copy + VectorE add).
    End-to-end precision ~3e-5 relative.
  - The same SBUF A chunk bytes feed both passes (the DoubleRow pair layout
    [p, 2, n] is just two adjacent src tiles of the chunk).
  - Then out^T = relu(W^T H^T + b) with an fp32 matmul; bias+relu fused in
    one ScalarE activation op.
  - DMA is spread over both HWDGE queues (sync: A stream, scalar: x/out)
    because DMA_DIRECT2D occupies the issuing engine for the whole transfer.
  - Host transposes/concats the 8 [128, 1250] outputs.
"""

import numpy as np
import ml_dtypes

N_NODES = 10000
N_EDGES = 640000
D = 128
NCORES = 8
NPC = N_NODES // NCORES            # 1250 dst nodes per core
DCOLS = NPC                        # dst cols per core (exact)
STILES = 79                        # ceil(10000 / 128) src tiles
SPAD = STILES * 128                # 10112 padded src rows
GROUPS = [(0, 512), (512, 512), (1024, 226)]   # dst col groups (PSUM banks)
ACH = 4                            # src tiles per A DMA chunk (even: 2 pairs)
XCH = 8                            # src tiles per x DMA chunk
LO_SCALE = 512.0                   # x_lo8 = fp8e4(512 * (x - bf16(x)))

BF16 = ml_dtypes.bfloat16
FP8 = ml_dtypes.float8_e4m3

_prog_cache = {}


def _build_program():
    from concourse import mybir
    import concourse.bacc as bacc
    import concourse.tile as tile

    # Bacc (not raw Bass): its compile pipeline legalizes multi-wait
    # instructions via event semaphores; raw Bass programs fail walrus
    # codegen with "Too many sync wait commands".
    nc = bacc.Bacc("TRN2", target_bir_lowering=False)

    xh = nc.dram_tensor("xh", [SPAD, D], mybir.dt.bfloat16, kind="ExternalInput")
    xl8 = nc.dram_tensor("xl8", [SPAD, D], mybir.dt.float8e4, kind="ExternalInput")
    A = nc.dram_tensor("A", [SPAD, DCOLS], mybir.dt.float8e4, kind="ExternalInput")
    W = nc.dram_tensor("W", [D, D], mybir.dt.float32, kind="ExternalInput")
    bcol = nc.dram_tensor("bcol", [D, 1], mybir.dt.float32, kind="ExternalInput")
    outT = nc.dram_tensor("outT", [D, DCOLS], mybir.dt.float32, kind="ExternalOutput")

    # HBM views with the src-tile index split out: row (s p) -> [p, s, cols]
    xh_r = xh.rearrange("(s p) d -> p s d", p=128)
    xl8_r = xl8.rearrange("(s p) d -> p s d", p=128)
    A_r = A.rearrange("(s p) d -> p s d", p=128)

    f32 = mybir.dt.float32
    Relu = mybir.ActivationFunctionType.Relu
    Copy = mybir.ActivationFunctionType.Copy
    DoubleRow = mybir.MatmulPerfMode.DoubleRow

    with tile.TileContext(nc) as tc:
        with (
            tc.tile_pool(name="xpool", bufs=1) as xpool,
            tc.tile_pool(name="apool", bufs=8) as apool,
            tc.tile_pool(name="cpool", bufs=1) as cpool,
            tc.tile_pool(name="hpool", bufs=2) as hpool,
            tc.tile_pool(name="opool", bufs=2) as opool,
            tc.tile_pool(name="pspool", bufs=1, space="PSUM") as pspool,
            tc.tile_pool(name="ps2pool", bufs=2, space="PSUM") as ps2pool,
        ):
            # ---- resident x tiles (scalar HWDGE queue, 8-tile chunks) ----
            xh_tiles = []   # per src tile: AP [128, 128] bf16
            xl_tiles = []   # per src tile: AP [128, 128] fp8
            xl_chunks = []  # per chunk: (chunk tile, c0, n) for pair slicing
            for ci, c0 in enumerate(range(0, STILES, XCH)):
                n = min(XCH, STILES - c0)
                th = xpool.tile([128, n, D], mybir.dt.bfloat16, tag=f"xh{ci}")
                nc.scalar.dma_start(out=th[:], in_=xh_r[:, c0 : c0 + n, :])
                tl = xpool.tile([128, n, D], mybir.dt.float8e4, tag=f"xl{ci}")
                nc.scalar.dma_start(out=tl[:], in_=xl8_r[:, c0 : c0 + n, :])
                for i in range(n):
                    xh_tiles.append(th[:, i, :])
                    xl_tiles.append(tl[:, i, :])
                xl_chunks.append((tl, c0, n))

            def xl_pair(s):
                # [128, 2, 128] fp8 lhsT for the DoubleRow pair (s, s+1);
                # XCH is even so pairs never straddle x chunks
                ci, i = s // XCH, s % XCH
                tl, c0, n = xl_chunks[ci]
                assert c0 + i + 2 <= c0 + n
                return tl[:, i : i + 2, :]

            # ---- constants (scalar queue) ----
            w_sb = cpool.tile([D, D], f32, tag="w")
            nc.scalar.dma_start(out=w_sb[:], in_=W[:, :])
            b_sb = cpool.tile([D, 1], f32, tag="b")
            nc.scalar.dma_start(out=b_sb[:], in_=bcol[:, :])

            # ---- phase 1: H^T[k, d] accumulation per col group ----
            ps_hi = []
            ps_lo = []
            for g, (off, wdt) in enumerate(GROUPS):
                ps_hi.append(pspool.tile([128, wdt], f32, tag=f"psh{g}", name=f"psh{g}"))
                ps_lo.append(pspool.tile([128, wdt], f32, tag=f"psl{g}", name=f"psl{g}"))

            NPAIR = STILES // 2  # 39 DoubleRow pairs; tile 78 handled singly

            def phase1(g):
                off, wdt = GROUPS[g]
                nhi = 0
                nlo = 0
                total_hi = STILES
                total_lo = NPAIR + 1
                for s0 in range(0, STILES, ACH):
                    n = min(ACH, STILES - s0)
                    at = apool.tile([128, n, wdt], mybir.dt.float8e4, tag="a")
                    nc.sync.dma_start(
                        out=at[:], in_=A_r[:, s0 : s0 + n, off : off + wdt]
                    )
                    # hi pass: bf16 x_hi stationary, fp8 A moving
                    for i in range(n):
                        nc.tensor.matmul(
                            out=ps_hi[g][:],
                            lhsT=xh_tiles[s0 + i][:],
                            rhs=at[:, i, :],
                            start=(nhi == 0),
                            stop=(nhi == total_hi - 1),
                        )
                        nhi += 1
                    # lo pass: fp8 DoubleRow over tile pairs of this chunk
                    for i in range(0, n - 1, 2):
                        nc.tensor.matmul(
                            out=ps_lo[g][:],
                            lhsT=xl_pair(s0 + i),
                            rhs=at[:, i : i + 2, :],
                            start=(nlo == 0),
                            stop=False,
                            perf_mode=DoubleRow,
                        )
                        nlo += 1
                    if n % 2 == 1:  # leftover single tile (s = 78)
                        nc.tensor.matmul(
                            out=ps_lo[g][:],
                            lhsT=xl_tiles[s0 + n - 1][:],
                            rhs=at[:, n - 1, :],
                            start=False,
                            stop=True,
                        )
                        nlo += 1

            def phase2(g):
                off, wdt = GROUPS[g]
                # hT = ps_hi + ps_lo / LO_SCALE
                lo_sc = hpool.tile([128, wdt], f32, tag="losc")
                nc.scalar.activation(
                    out=lo_sc[:], in_=ps_lo[g][:], func=Copy, scale=1.0 / LO_SCALE
                )
                hT = hpool.tile([128, wdt], f32, tag="hT")
                nc.vector.tensor_add(out=hT[:], in0=lo_sc[:], in1=ps_hi[g][:])
                po = ps2pool.tile([128, wdt], f32, tag="po")
                nc.tensor.matmul(out=po[:], lhsT=w_sb[:], rhs=hT[:], start=True, stop=True)
                ot = opool.tile([128, wdt], f32, tag="ot")
                nc.scalar.activation(out=ot[:], in_=po[:], func=Relu, bias=b_sb[:], scale=1.0)
                nc.scalar.dma_start(out=outT[:, off : off + wdt], in_=ot[:])

            # order: ph1(0), ph1(1), ph2(0), ph1(2), ph2(1), ph2(2)
            phase1(0)
            phase1(1)
            phase2(0)
            phase1(2)
            phase2(1)
            phase2(2)

    nc.finalize()
    return nc


def _host_preprocess(x, src, dst, W, b):
    x = np.asarray(x, dtype=np.float32)
    xh32 = x.astype(BF16).astype(np.float32)
    xh = np.zeros((SPAD, D), dtype=BF16)
    xh[:N_NODES] = xh32.astype(BF16)
    xl8 = np.zeros((SPAD, D), dtype=FP8)
    xl8[:N_NODES] = ((x - xh32) * LO_SCALE).astype(FP8)

    src = np.asarray(src).astype(np.int64)
    dst = np.asarray(dst).astype(np.int64)

    A_mats = []
    for c in range(NCORES):
        lo, hi = c * NPC, (c + 1) * NPC
        m = (dst >= lo) & (dst < hi)
        idx = src[m] * DCOLS + (dst[m] - lo)
        cnt = np.bincount(idx, minlength=SPAD * DCOLS)
        assert cnt.max() <= 16, "count too large for exact fp8e4"
        A_mats.append(cnt.reshape(SPAD, DCOLS).astype(FP8))

    Wf = np.asarray(W, dtype=np.float32)
    bc = np.asarray(b, dtype=np.float32).reshape(D, 1)
    return xh, xl8, A_mats, Wf, bc


def kernel(x, src, dst, W, b):
    from concourse.bass_utils import run_bass_kernel_spmd

    xh, xl8, A_mats, Wf, bc = _host_preprocess(x, src, dst, W, b)

    if "nc" not in _prog_cache:
        _prog_cache["nc"] = _build_program()
    nc = _prog_cache["nc"]

    in_maps = [
        {"xh": xh, "xl8": xl8, "A": A_mats[c], "W": Wf, "bcol": bc}
        for c in range(NCORES)
    ]
    res = run_bass_kernel_spmd(nc, in_maps, core_ids=list(range(NCORES)))

    out = np.empty((N_NODES, D), dtype=np.float32)
    for c in range(NCORES):
        outT = res.results[c]["outT"]  # [128, 1250]
        out[c * NPC : (c + 1) * NPC] = outT.T
    return out
